# revision 7
# baseline (speedup 1.0000x reference)
"""MoE layer (E=8 experts, top-2, capacity 2560) on 8 TRN2 NeuronCores.

Strategy (expert-parallel, as suggested by the sharding hint):
  - Host: gating (logits / top-k / softmax / ranks) mirrored with the exact
    same jax ops as the reference so routing decisions match bitwise.
  - Host dispatch: gather each expert's kept tokens into a dense, padded,
    pre-transposed activation buffer XT (H, M); one expert per core.
  - Device (per core): YT = W2^T gelu(W1^T XT + b1) + b2, all matmuls in
    fp32r (1 cycle/row on the PE when the moving dim >= 256).
  - Host combine: scatter-add gate-weighted rows back into the output.

The device kernel is two phases, both producing transposed outputs so no
on-device transposes are needed:
  phase 1: HT[i, m] = gelu(sum_k W1[k, i]^T @ XT[k, m] + b1[i])  -> DRAM
  phase 2: YT[h, m] = sum_k W2[k, h]^T @ HT[k, m] + b2[h]        -> DRAM
"""

import os

import numpy as np

B, S, H, I, E = 4, 2048, 1024, 4096, 8
TOP_K = 2
T = B * S
CAPACITY = int(1.25 * T * TOP_K / E)  # 2560
P = 128
KT = H // P  # 8 k-tiles over H
IT = I // P  # 32 i-tiles over I
CHUNK = 512  # token-chunk (PSUM bank = 512 fp32)

# Set by kernel() after a profiled run (test harness convenience).
LAST_EXEC_TIME_NS = None


def _build_bass(M: int):
    import concourse.bass as bass  # noqa: F401
    import concourse.mybir as mybir
    from concourse import bacc
    from concourse.tile import TileContext

    f32 = mybir.dt.float32
    f32r = mybir.dt.float32r
    IDENT = mybir.ActivationFunctionType.Identity
    # CoreSim doesn't implement Gelu; allow swapping it out for wiring tests.
    if os.environ.get("MOE_SIM_IDENTITY"):
        GELU = IDENT
    else:
        GELU = mybir.ActivationFunctionType.Gelu

    n_chunks = M // CHUNK
    assert M % CHUNK == 0

    # Bacc (not raw Bass): its lowering splits multi-sem waits that the
    # walrus codegen can't encode on a single instruction.
    nc = bacc.Bacc("TRN2", target_bir_lowering=False, debug=False, num_devices=E)
    # float32r is bit-identical to float32 (np dtype float32); typing the
    # whole matmul input chain as f32r satisfies the BIR verifier's
    # "rounded to FP32r" producer check while running the PE at 1 cyc/row.
    xt = nc.declare_dram_parameter("xt", [H, M], f32r, isOutput=False)
    w1 = nc.declare_dram_parameter("w1", [H, I], f32r, isOutput=False)
    b1 = nc.declare_dram_parameter("b1", [I], f32, isOutput=False)
    w2 = nc.declare_dram_parameter("w2", [I, H], f32r, isOutput=False)
    b2 = nc.declare_dram_parameter("b2", [H], f32, isOutput=False)
    yt = nc.declare_dram_parameter("yt", [H, M], f32, isOutput=True)
    ht = nc.dram_tensor("ht", [I, M], f32r)  # internal scratch

    with TileContext(nc) as tc:
        # ---------------- phase 1: HT = gelu(W1^T X + b1) ----------------
        with (
            tc.tile_pool(name="w1_pool", bufs=1) as w1_pool,
            tc.tile_pool(name="b1_pool", bufs=1) as b1_pool,
            tc.tile_pool(name="xt_pool", bufs=2 * KT) as xt_pool,
            tc.tile_pool(name="hs_pool", bufs=4) as hs_pool,
            tc.tile_pool(name="ps1_pool", bufs=4, space="PSUM") as ps1_pool,
        ):
            b1_sb = b1_pool.tile([P, IT], f32, tag="b1")
            nc.sync.dma_start(out=b1_sb, in_=b1[:].rearrange("(a p) -> p a", p=P))

            w1_tiles = []
            for k in range(KT):
                w1_t = w1_pool.tile([P, I], f32r, name=f"w1_{k}", tag=f"w1_{k}")
                nc.sync.dma_start(out=w1_t, in_=w1[k * P : (k + 1) * P, :])
                w1_tiles.append(w1_t)

            for c in range(n_chunks):
                cs = slice(c * CHUNK, (c + 1) * CHUNK)
                xt_tiles = []
                for k in range(KT):
                    xt_t = xt_pool.tile([P, CHUNK], f32r, name=f"xt_{k}", tag="xt")
                    nc.sync.dma_start(out=xt_t, in_=xt[k * P : (k + 1) * P, cs])
                    xt_tiles.append(xt_t)
                for i in range(IT):
                    ps = ps1_pool.tile([P, CHUNK], f32, name="ps", tag="ps1")
                    for k in range(KT):
                        nc.tensor.matmul(
                            ps,
                            lhsT=w1_tiles[k][:, i * P : (i + 1) * P],
                            rhs=xt_tiles[k],
                            start=(k == 0),
                            stop=(k == KT - 1),
                        )
                    hs = hs_pool.tile([P, CHUNK], f32r, name="hs", tag="hs")
                    nc.scalar.activation(hs, ps, GELU, bias=b1_sb[:, i : i + 1])
                    nc.sync.dma_start(out=ht[i * P : (i + 1) * P, cs], in_=hs)

        # ---------------- phase 2: YT = W2^T HT + b2 ----------------
        with (
            tc.tile_pool(name="w2_pool", bufs=1) as w2_pool,
            tc.tile_pool(name="b2_pool", bufs=1) as b2_pool,
            tc.tile_pool(name="ht_pool", bufs=16) as ht_pool,
            tc.tile_pool(name="ys_pool", bufs=4) as ys_pool,
            tc.tile_pool(name="ps2_pool", bufs=1, space="PSUM") as ps2_pool,
        ):
            b2_sb = b2_pool.tile([P, KT], f32, tag="b2")
            nc.sync.dma_start(out=b2_sb, in_=b2[:].rearrange("(a p) -> p a", p=P))

            w2_tiles = []
            for k in range(IT):
                w2_t = w2_pool.tile([P, H], f32r, name=f"w2_{k}", tag=f"w2_{k}")
                nc.sync.dma_start(out=w2_t, in_=w2[k * P : (k + 1) * P, :])
                w2_tiles.append(w2_t)

            for c in range(n_chunks):
                cs = slice(c * CHUNK, (c + 1) * CHUNK)
                ps_tiles = [
                    ps2_pool.tile([P, CHUNK], f32, name=f"psy_{h}", tag=f"ps2_{h}")
                    for h in range(KT)
                ]
                for k in range(IT):
                    ht_t = ht_pool.tile([P, CHUNK], f32r, name="ht_t", tag="ht")
                    nc.sync.dma_start(out=ht_t, in_=ht[k * P : (k + 1) * P, cs])
                    for h in range(KT):
                        nc.tensor.matmul(
                            ps_tiles[h],
                            lhsT=w2_tiles[k][:, h * P : (h + 1) * P],
                            rhs=ht_t,
                            start=(k == 0),
                            stop=(k == IT - 1),
                        )
                for h in range(KT):
                    ys = ys_pool.tile([P, CHUNK], f32, name="ys", tag="ys")
                    # Alternate ACT/DVE so the 8 end-of-chunk PSUM drains
                    # don't serialize on one engine.
                    if h % 2 == 0:
                        nc.vector.tensor_scalar_add(ys, ps_tiles[h], b2_sb[:, h : h + 1])
                    else:
                        nc.scalar.activation(ys, ps_tiles[h], IDENT, bias=b2_sb[:, h : h + 1])
                    nc.sync.dma_start(out=yt[h * P : (h + 1) * P, cs], in_=ys)

    nc.compile()
    return nc


def kernel(hidden_states, gate_w, w1, b1, w2, b2):
    global LAST_EXEC_TIME_NS
    import jax
    import jax.numpy as jnp

    from concourse.bass_utils import run_bass_kernel_spmd

    # ---- gating: mirror the reference's jax ops so routing matches ----
    hs_j = jnp.asarray(hidden_states)
    xf_j = hs_j.reshape(T, H)
    logits = xf_j @ jnp.asarray(gate_w).T  # (T, E)
    top_vals, top_idx = jax.lax.top_k(logits, TOP_K)  # (T, K)
    gates = jax.nn.softmax(top_vals, axis=-1)  # (T, K)

    probs = jax.nn.softmax(logits, axis=-1)
    mean_probs = probs.mean(axis=0)
    routing_probs = (probs > 0).astype(probs.dtype).mean(axis=0)
    aux_loss = np.asarray(E * (mean_probs * routing_probs).sum())

    xf = np.asarray(xf_j, dtype=np.float32)
    top_idx_np = np.asarray(top_idx)
    gates_np = np.asarray(gates, dtype=np.float32)
    w1_np = np.asarray(w1, dtype=np.float32)
    b1_np = np.asarray(b1, dtype=np.float32)
    w2_np = np.asarray(w2, dtype=np.float32)
    b2_np = np.asarray(b2, dtype=np.float32)

    # ---- dispatch: first CAPACITY tokens (row-major order) per (k, e) ----
    rows_per_expert = []  # e -> list over k of kept token indices
    for e in range(E):
        per_k = []
        for k in range(TOP_K):
            tok = np.nonzero(top_idx_np[:, k] == e)[0]
            per_k.append(tok[:CAPACITY])
        rows_per_expert.append(per_k)

    max_rows = max(sum(len(t) for t in per_k) for per_k in rows_per_expert)
    M = max(CHUNK, ((max_rows + CHUNK - 1) // CHUNK) * CHUNK)

    in_maps = []
    for e in range(E):
        rows = np.concatenate(rows_per_expert[e])
        xt_e = np.zeros((H, M), dtype=np.float32)
        if len(rows):
            xt_e[:, : len(rows)] = xf[rows].T
        in_maps.append(
            {
                "xt": xt_e,
                "w1": np.ascontiguousarray(w1_np[e]),
                "b1": np.ascontiguousarray(b1_np[e]),
                "w2": np.ascontiguousarray(w2_np[e]),
                "b2": np.ascontiguousarray(b2_np[e]),
            }
        )

    # ---- device FFN ----
    nc = _build_bass(M)
    trace = bool(os.environ.get("MOE_KERNEL_TRACE"))
    res = run_bass_kernel_spmd(nc, in_maps, core_ids=list(range(E)), trace=trace)
    LAST_EXEC_TIME_NS = res.exec_time_ns

    # ---- combine ----
    out = np.zeros((T, H), dtype=np.float32)
    for e in range(E):
        yt_e = np.asarray(res.results[e]["yt"])  # (H, M)
        y_e = np.ascontiguousarray(yt_e.T)  # (M, H)
        ofs = 0
        for k in range(TOP_K):
            rows = rows_per_expert[e][k]
            n = len(rows)
            if n:
                out[rows] += gates_np[rows, k][:, None] * y_e[ofs : ofs + n]
            ofs += n

    return out.reshape(B, S, H), np.float32(aux_loss)


# revision 8
# speedup vs baseline: 1.2196x; 1.2196x over previous
"""MoE layer (E=8 experts, top-2, capacity 2560) on 8 TRN2 NeuronCores.

Strategy (expert-parallel, as suggested by the sharding hint):
  - Host: gating (logits / top-k / softmax / ranks) mirrored with the exact
    same jax ops as the reference so routing decisions match bitwise.
  - Host dispatch: gather each expert's kept tokens into a dense, padded,
    pre-transposed activation buffer XT (H, M); one expert per core.
  - Device (per core): YT = W2^T gelu(W1^T XT + b1) + b2, all matmuls in
    fp32r (1 cycle/row on the PE when the moving dim >= 256).
  - Host combine: scatter-add gate-weighted rows back into the output.

The device kernel is two phases, both producing transposed outputs so no
on-device transposes are needed:
  phase 1: HT[i, m] = gelu(sum_k W1[k, i]^T @ XT[k, m] + b1[i])  -> DRAM
  phase 2: YT[h, m] = sum_k W2[k, h]^T @ HT[k, m] + b2[h]        -> DRAM

Schedule notes (from NTFF profiling):
  - Token chunks are sized {512,384,256} so M only rounds up to 128 rows
    (fp32r needs a moving dim >= 256 to run at 1 cyc/row).
  - Phase-1 chunk 0 runs k-outer over 4-PSUM-bank i-groups with W1/XT DMAs
    interleaved per-k, so the PE starts ~7us in instead of waiting ~50us
    for the whole W1 load; later chunks run i-outer (gelu drains pipeline
    against the accumulation of the next i-tile).
  - Phase-2 W2 tile loads are interleaved into chunk 0's k-loop, removing
    the ~47us phase-transition stall a monolithic W2 load causes.
"""

import os

import numpy as np

B, S, H, I, E = 4, 2048, 1024, 4096, 8
TOP_K = 2
T = B * S
CAPACITY = int(1.25 * T * TOP_K / E)  # 2560
P = 128
KT = H // P  # 8 k-tiles over H
IT = I // P  # 32 i-tiles over I

# Set by kernel() after a profiled run (test harness convenience).
LAST_EXEC_TIME_NS = None


def _plan_chunks(M):
    """Split M (multiple of 128, >= 256) into chunks of 256..512 rows."""
    chunks = []
    while M > 768:
        chunks.append(512)
        M -= 512
    chunks += {256: [256], 384: [384], 512: [512], 640: [384, 256], 768: [512, 256]}[M]
    return chunks


def _build_bass(M: int):
    import concourse.bass as bass  # noqa: F401
    import concourse.mybir as mybir
    from concourse import bacc
    from concourse.tile import TileContext

    f32 = mybir.dt.float32
    f32r = mybir.dt.float32r
    IDENT = mybir.ActivationFunctionType.Identity
    # CoreSim doesn't implement Gelu; allow swapping it out for wiring tests.
    if os.environ.get("MOE_SIM_IDENTITY"):
        GELU = IDENT
    else:
        GELU = mybir.ActivationFunctionType.Gelu

    chunks = _plan_chunks(M)
    offs = [sum(chunks[:c]) for c in range(len(chunks))]

    # Bacc (not raw Bass): its lowering splits multi-sem waits that the
    # walrus codegen can't encode on a single instruction.
    nc = bacc.Bacc("TRN2", target_bir_lowering=False, debug=False, num_devices=E)
    # float32r is bit-identical to float32 (np dtype float32); typing the
    # whole matmul input chain as f32r satisfies the BIR verifier's
    # "rounded to FP32r" producer check while running the PE at 1 cyc/row.
    xt = nc.declare_dram_parameter("xt", [H, M], f32r, isOutput=False)
    w1 = nc.declare_dram_parameter("w1", [H, I], f32r, isOutput=False)
    b1 = nc.declare_dram_parameter("b1", [I], f32, isOutput=False)
    w2 = nc.declare_dram_parameter("w2", [I, H], f32r, isOutput=False)
    b2 = nc.declare_dram_parameter("b2", [H], f32, isOutput=False)
    yt = nc.declare_dram_parameter("yt", [H, M], f32, isOutput=True)
    ht = nc.dram_tensor("ht", [I, M], f32r)  # internal scratch

    with TileContext(nc) as tc:
        # ---------------- phase 1: HT = gelu(W1^T X + b1) ----------------
        with (
            tc.tile_pool(name="w1_pool", bufs=1) as w1_pool,
            tc.tile_pool(name="b1_pool", bufs=1) as b1_pool,
            tc.tile_pool(name="xt_pool", bufs=2 * KT) as xt_pool,
            tc.tile_pool(name="hs_pool", bufs=4) as hs_pool,
            tc.tile_pool(name="psA_pool", bufs=1, space="PSUM") as psA_pool,
            tc.tile_pool(name="psB_pool", bufs=4, space="PSUM") as psB_pool,
        ):
            b1_sb = b1_pool.tile([P, IT], f32, tag="b1")
            nc.sync.dma_start(out=b1_sb, in_=b1[:].rearrange("(a p) -> p a", p=P))

            # Chunk 0: interleave W1 k-tiles with XT k-tiles so accumulation
            # over k can begin as soon as the first pair lands.
            w1_tiles = []
            xt0_tiles = []
            c0 = chunks[0]
            for k in range(KT):
                w1_t = w1_pool.tile([P, I], f32r, name=f"w1_{k}", tag=f"w1_{k}")
                nc.sync.dma_start(out=w1_t, in_=w1[k * P : (k + 1) * P, :])
                w1_tiles.append(w1_t)
                xt_t = xt_pool.tile([P, c0], f32r, name=f"xt_{k}", tag="xt")
                nc.sync.dma_start(out=xt_t, in_=xt[k * P : (k + 1) * P, 0:c0])
                xt0_tiles.append(xt_t)

            def gelu_store(ps, i, c, cs):
                hs = hs_pool.tile([P, chunks[c]], f32r, name="hs", tag="hs")
                nc.scalar.activation(hs, ps, GELU, bias=b1_sb[:, i : i + 1])
                nc.sync.dma_start(out=ht[i * P : (i + 1) * P, cs], in_=hs)

            # Chunk 0 compute: k-outer over 4-bank i-groups (DMA-bound while
            # W1 streams in, so the group-end gelu drains hide completely).
            cs0 = slice(0, c0)
            for g in range(IT // 4):
                ps_g = [
                    psA_pool.tile([P, c0], f32, name=f"psA_{j}", tag=f"psA_{j}")
                    for j in range(4)
                ]
                for k in range(KT):
                    for j in range(4):
                        i = 4 * g + j
                        nc.tensor.matmul(
                            ps_g[j],
                            lhsT=w1_tiles[k][:, i * P : (i + 1) * P],
                            rhs=xt0_tiles[k],
                            start=(k == 0),
                            stop=(k == KT - 1),
                        )
                for j in range(4):
                    gelu_store(ps_g[j], 4 * g + j, 0, cs0)

            # Chunks 1+: i-outer, gelu pipelined against the next i-tile.
            for c in range(1, len(chunks)):
                cw = chunks[c]
                cs = slice(offs[c], offs[c] + cw)
                xt_tiles = []
                for k in range(KT):
                    xt_t = xt_pool.tile([P, cw], f32r, name=f"xt_{k}", tag="xt")
                    nc.sync.dma_start(out=xt_t, in_=xt[k * P : (k + 1) * P, cs])
                    xt_tiles.append(xt_t)
                for i in range(IT):
                    ps = psB_pool.tile([P, cw], f32, name="ps", tag="psB")
                    for k in range(KT):
                        nc.tensor.matmul(
                            ps,
                            lhsT=w1_tiles[k][:, i * P : (i + 1) * P],
                            rhs=xt_tiles[k],
                            start=(k == 0),
                            stop=(k == KT - 1),
                        )
                    gelu_store(ps, i, c, cs)

        # ---------------- phase 2: YT = W2^T HT + b2 ----------------
        with (
            tc.tile_pool(name="w2_pool", bufs=1) as w2_pool,
            tc.tile_pool(name="b2_pool", bufs=1) as b2_pool,
            tc.tile_pool(name="ht_pool", bufs=16) as ht_pool,
            tc.tile_pool(name="ys_pool", bufs=4) as ys_pool,
            tc.tile_pool(name="ps2_pool", bufs=1, space="PSUM") as ps2_pool,
        ):
            b2_sb = b2_pool.tile([P, KT], f32, tag="b2")
            nc.sync.dma_start(out=b2_sb, in_=b2[:].rearrange("(a p) -> p a", p=P))

            w2_tiles = [None] * IT

            for c in range(len(chunks)):
                cw = chunks[c]
                cs = slice(offs[c], offs[c] + cw)
                ps_tiles = [
                    ps2_pool.tile([P, cw], f32, name=f"psy_{h}", tag=f"ps2_{h}")
                    for h in range(KT)
                ]
                for k in range(IT):
                    if c == 0:
                        # Interleave the W2 load with chunk 0's HT stream so
                        # phase 2 starts as soon as the first tiles land.
                        w2_t = w2_pool.tile([P, H], f32r, name=f"w2_{k}", tag=f"w2_{k}")
                        nc.sync.dma_start(out=w2_t, in_=w2[k * P : (k + 1) * P, :])
                        w2_tiles[k] = w2_t
                    ht_t = ht_pool.tile([P, cw], f32r, name="ht_t", tag="ht")
                    nc.sync.dma_start(out=ht_t, in_=ht[k * P : (k + 1) * P, cs])
                    for h in range(KT):
                        nc.tensor.matmul(
                            ps_tiles[h],
                            lhsT=w2_tiles[k][:, h * P : (h + 1) * P],
                            rhs=ht_t,
                            start=(k == 0),
                            stop=(k == IT - 1),
                        )
                for h in range(KT):
                    ys = ys_pool.tile([P, cw], f32, name="ys", tag="ys")
                    # Alternate ACT/DVE so the end-of-chunk PSUM drains don't
                    # serialize on one engine.
                    if h % 2 == 0:
                        nc.vector.tensor_scalar_add(ys, ps_tiles[h], b2_sb[:, h : h + 1])
                    else:
                        nc.scalar.activation(ys, ps_tiles[h], IDENT, bias=b2_sb[:, h : h + 1])
                    nc.sync.dma_start(out=yt[h * P : (h + 1) * P, cs], in_=ys)

    nc.compile()
    return nc


def kernel(hidden_states, gate_w, w1, b1, w2, b2):
    global LAST_EXEC_TIME_NS
    import jax
    import jax.numpy as jnp

    from concourse.bass_utils import run_bass_kernel_spmd

    # ---- gating: mirror the reference's jax ops so routing matches ----
    hs_j = jnp.asarray(hidden_states)
    xf_j = hs_j.reshape(T, H)
    logits = xf_j @ jnp.asarray(gate_w).T  # (T, E)
    top_vals, top_idx = jax.lax.top_k(logits, TOP_K)  # (T, K)
    gates = jax.nn.softmax(top_vals, axis=-1)  # (T, K)

    probs = jax.nn.softmax(logits, axis=-1)
    mean_probs = probs.mean(axis=0)
    routing_probs = (probs > 0).astype(probs.dtype).mean(axis=0)
    aux_loss = np.asarray(E * (mean_probs * routing_probs).sum())

    xf = np.asarray(xf_j, dtype=np.float32)
    top_idx_np = np.asarray(top_idx)
    gates_np = np.asarray(gates, dtype=np.float32)
    w1_np = np.asarray(w1, dtype=np.float32)
    b1_np = np.asarray(b1, dtype=np.float32)
    w2_np = np.asarray(w2, dtype=np.float32)
    b2_np = np.asarray(b2, dtype=np.float32)

    # ---- dispatch: first CAPACITY tokens (row-major order) per (k, e) ----
    rows_per_expert = []  # e -> list over k of kept token indices
    for e in range(E):
        per_k = []
        for k in range(TOP_K):
            tok = np.nonzero(top_idx_np[:, k] == e)[0]
            per_k.append(tok[:CAPACITY])
        rows_per_expert.append(per_k)

    max_rows = max(sum(len(t) for t in per_k) for per_k in rows_per_expert)
    M = max(256, ((max_rows + P - 1) // P) * P)

    in_maps = []
    for e in range(E):
        rows = np.concatenate(rows_per_expert[e])
        xt_e = np.zeros((H, M), dtype=np.float32)
        if len(rows):
            xt_e[:, : len(rows)] = xf[rows].T
        in_maps.append(
            {
                "xt": xt_e,
                "w1": np.ascontiguousarray(w1_np[e]),
                "b1": np.ascontiguousarray(b1_np[e]),
                "w2": np.ascontiguousarray(w2_np[e]),
                "b2": np.ascontiguousarray(b2_np[e]),
            }
        )

    # ---- device FFN ----
    nc = _build_bass(M)
    trace = bool(os.environ.get("MOE_KERNEL_TRACE"))
    res = run_bass_kernel_spmd(nc, in_maps, core_ids=list(range(E)), trace=trace)
    LAST_EXEC_TIME_NS = res.exec_time_ns

    # ---- combine ----
    out = np.zeros((T, H), dtype=np.float32)
    for e in range(E):
        yt_e = np.asarray(res.results[e]["yt"])  # (H, M)
        y_e = np.ascontiguousarray(yt_e.T)  # (M, H)
        ofs = 0
        for k in range(TOP_K):
            rows = rows_per_expert[e][k]
            n = len(rows)
            if n:
                out[rows] += gates_np[rows, k][:, None] * y_e[ofs : ofs + n]
            ofs += n

    return out.reshape(B, S, H), np.float32(aux_loss)


# revision 12
# speedup vs baseline: 1.2287x; 1.0075x over previous
"""MoE layer (E=8 experts, top-2, capacity 2560) on 8 TRN2 NeuronCores.

Strategy (expert-parallel, as suggested by the sharding hint):
  - Host: gating (logits / top-k / softmax / ranks) mirrored with the exact
    same jax ops as the reference so routing decisions match bitwise.
  - Host dispatch: gather each expert's kept tokens into a dense, padded,
    pre-transposed activation buffer XT (H, M); one expert per core.
  - Device (per core): YT = W2^T gelu(W1^T XT + b1) + b2, all matmuls in
    fp32r (1 cycle/row on the PE when the moving dim >= 256).
  - Host combine: scatter-add gate-weighted rows back into the output.

The device kernel is two phases, both producing transposed outputs so no
on-device transposes are needed:
  phase 1: HT[i, m] = gelu(sum_k W1[k, i]^T @ XT[k, m] + b1[i])  -> DRAM
  phase 2: YT[h, m] = sum_k W2[k, h]^T @ HT[k, m] + b2[h]        -> DRAM

Schedule notes (from NTFF profiling):
  - Token chunks are sized {512,384,256} so M only rounds up to 128 rows
    (fp32r needs a moving dim >= 256 to run at 1 cyc/row).
  - Phase-1 chunk 0 runs k-outer over 4-PSUM-bank i-groups with W1/XT DMAs
    interleaved per-k, so the PE starts ~7us in instead of waiting ~50us
    for the whole W1 load; later chunks run i-outer (gelu drains pipeline
    against the accumulation of the next i-tile).
  - Phase-2 W2 tile loads are interleaved into chunk 0's k-loop, removing
    the ~47us phase-transition stall a monolithic W2 load causes.
"""

import os

import numpy as np

B, S, H, I, E = 4, 2048, 1024, 4096, 8
TOP_K = 2
T = B * S
CAPACITY = int(1.25 * T * TOP_K / E)  # 2560
P = 128
KT = H // P  # 8 k-tiles over H
IT = I // P  # 32 i-tiles over I

# Set by kernel() after a profiled run (test harness convenience).
LAST_EXEC_TIME_NS = None


def _plan_chunks(M):
    """Split M (multiple of 128, >= 256) into chunks of 256..512 rows."""
    chunks = []
    while M > 768:
        chunks.append(512)
        M -= 512
    chunks += {256: [256], 384: [384], 512: [512], 640: [384, 256], 768: [512, 256]}[M]
    return chunks


def _build_bass(M: int):
    import concourse.bass as bass  # noqa: F401
    import concourse.mybir as mybir
    from concourse import bacc
    from concourse.tile import TileContext

    f32 = mybir.dt.float32
    f32r = mybir.dt.float32r
    IDENT = mybir.ActivationFunctionType.Identity
    # CoreSim doesn't implement Gelu; allow swapping it out for wiring tests.
    if os.environ.get("MOE_SIM_IDENTITY"):
        GELU = IDENT
    else:
        GELU = mybir.ActivationFunctionType.Gelu

    chunks = _plan_chunks(M)
    offs = [sum(chunks[:c]) for c in range(len(chunks))]

    # Bacc (not raw Bass): its lowering splits multi-sem waits that the
    # walrus codegen can't encode on a single instruction.
    nc = bacc.Bacc("TRN2", target_bir_lowering=False, debug=False, num_devices=E)
    # float32r is bit-identical to float32 (np dtype float32); typing the
    # whole matmul input chain as f32r satisfies the BIR verifier's
    # "rounded to FP32r" producer check while running the PE at 1 cyc/row.
    xt = nc.declare_dram_parameter("xt", [H, M], f32r, isOutput=False)
    w1 = nc.declare_dram_parameter("w1", [H, I], f32r, isOutput=False)
    b1 = nc.declare_dram_parameter("b1", [I], f32, isOutput=False)
    w2 = nc.declare_dram_parameter("w2", [I, H], f32r, isOutput=False)
    b2 = nc.declare_dram_parameter("b2", [H], f32, isOutput=False)
    yt = nc.declare_dram_parameter("yt", [H, M], f32, isOutput=True)
    ht = nc.dram_tensor("ht", [I, M], f32r)  # internal scratch

    GW = 4 * P  # W1 column-slice width = one 4-i-tile PSUM group

    with TileContext(nc) as tc:
        # w2_head outlives phase 1: the first 8 W2 k-tiles stream in during
        # phase 1's DMA slack so phase 2 starts with weights in hand.
        with tc.tile_pool(name="w2h_pool", bufs=1) as w2h_pool:
            w2h_tiles = [
                w2h_pool.tile([P, H], f32r, name=f"w2h_{k}", tag=f"w2h_{k}")
                for k in range(KT)
            ]
            # ---------------- phase 1: HT = gelu(W1^T X + b1) ----------------
            with (
                tc.tile_pool(name="w1_pool", bufs=1) as w1_pool,
                tc.tile_pool(name="b1_pool", bufs=1) as b1_pool,
                tc.tile_pool(name="xt_pool", bufs=2 * KT) as xt_pool,
                tc.tile_pool(name="hs_pool", bufs=4) as hs_pool,
                tc.tile_pool(name="psA_pool", bufs=1, space="PSUM") as psA_pool,
                tc.tile_pool(name="psB_pool", bufs=4, space="PSUM") as psB_pool,
            ):
                b1_sb = b1_pool.tile([P, IT], f32, tag="b1")
                nc.sync.dma_start(out=b1_sb, in_=b1[:].rearrange("(a p) -> p a", p=P))

                # Chunk 0: W1 arrives in GW-column slices, interleaved with
                # the XT k-tiles, issued in exactly the order the k-outer
                # group loop consumes them — the first matmul can start
                # after ~0.5 MB instead of ~19 MB.
                c0 = chunks[0]
                w1_tiles = [
                    w1_pool.tile([P, I], f32r, name=f"w1_{k}", tag=f"w1_{k}")
                    for k in range(KT)
                ]
                xt0_tiles = []
                for k in range(KT):
                    xt_t = xt_pool.tile([P, c0], f32r, name=f"xt_{k}", tag="xt")
                    nc.sync.dma_start(out=xt_t, in_=xt[k * P : (k + 1) * P, 0:c0])
                    xt0_tiles.append(xt_t)
                    nc.sync.dma_start(
                        out=w1_tiles[k][:, 0:GW], in_=w1[k * P : (k + 1) * P, 0:GW]
                    )
                for g in range(1, IT // 4):
                    gs = slice(g * GW, (g + 1) * GW)
                    for k in range(KT):
                        nc.sync.dma_start(
                            out=w1_tiles[k][:, gs], in_=w1[k * P : (k + 1) * P, gs]
                        )

                def gelu_store(ps, i, c, cs):
                    hs = hs_pool.tile([P, chunks[c]], f32r, name="hs", tag="hs")
                    nc.scalar.activation(hs, ps, GELU, bias=b1_sb[:, i : i + 1])
                    nc.sync.dma_start(out=ht[i * P : (i + 1) * P, cs], in_=hs)

                # Chunk 0 compute: k-outer over 4-bank i-groups (paced by the
                # W1 stream; group-end gelu drains hide under the DMA wait).
                cs0 = slice(0, c0)
                for g in range(IT // 4):
                    ps_g = [
                        psA_pool.tile([P, c0], f32, name=f"psA_{j}", tag=f"psA_{j}")
                        for j in range(4)
                    ]
                    for k in range(KT):
                        for j in range(4):
                            i = 4 * g + j
                            nc.tensor.matmul(
                                ps_g[j],
                                lhsT=w1_tiles[k][:, i * P : (i + 1) * P],
                                rhs=xt0_tiles[k],
                                start=(k == 0),
                                stop=(k == KT - 1),
                            )
                    for j in range(4):
                        gelu_store(ps_g[j], 4 * g + j, 0, cs0)

                # Chunks 1+: i-outer, gelu pipelined against the next i-tile.
                for c in range(1, len(chunks)):
                    cw = chunks[c]
                    cs = slice(offs[c], offs[c] + cw)
                    xt_tiles = []
                    for k in range(KT):
                        xt_t = xt_pool.tile([P, cw], f32r, name=f"xt_{k}", tag="xt")
                        nc.sync.dma_start(out=xt_t, in_=xt[k * P : (k + 1) * P, cs])
                        xt_tiles.append(xt_t)
                    if c == min(2, len(chunks) - 1):
                        # Phase-1 DMA has ~2x slack per chunk by now; pull the
                        # first 8 W2 k-tiles in ahead of the phase switch.
                        for k in range(KT):
                            nc.sync.dma_start(
                                out=w2h_tiles[k], in_=w2[k * P : (k + 1) * P, :]
                            )
                    for i in range(IT):
                        ps = psB_pool.tile([P, cw], f32, name="ps", tag="psB")
                        for k in range(KT):
                            nc.tensor.matmul(
                                ps,
                                lhsT=w1_tiles[k][:, i * P : (i + 1) * P],
                                rhs=xt_tiles[k],
                                start=(k == 0),
                                stop=(k == KT - 1),
                            )
                        gelu_store(ps, i, c, cs)

                if len(chunks) == 1:
                    for k in range(KT):
                        nc.sync.dma_start(
                            out=w2h_tiles[k], in_=w2[k * P : (k + 1) * P, :]
                        )

            # ---------------- phase 2: YT = W2^T HT + b2 ----------------
            with (
                tc.tile_pool(name="w2_pool", bufs=1) as w2_pool,
                tc.tile_pool(name="b2_pool", bufs=1) as b2_pool,
                tc.tile_pool(name="ht_pool", bufs=24) as ht_pool,
                tc.tile_pool(name="ys_pool", bufs=6) as ys_pool,
                tc.tile_pool(name="ps2_pool", bufs=1, space="PSUM") as ps2_pool,
            ):
                b2_sb = b2_pool.tile([P, KT], f32, tag="b2")
                nc.sync.dma_start(out=b2_sb, in_=b2[:].rearrange("(a p) -> p a", p=P))

                w2_tiles = list(w2h_tiles) + [None] * (IT - KT)

                for c in range(len(chunks)):
                    cw = chunks[c]
                    cs = slice(offs[c], offs[c] + cw)
                    ps_tiles = [
                        ps2_pool.tile([P, cw], f32, name=f"psy_{h}", tag=f"ps2_{h}")
                        for h in range(KT)
                    ]
                    for k in range(IT):
                        if c == 0 and k >= KT:
                            # Interleave the rest of the W2 load with chunk
                            # 0's HT stream (k < KT came in during phase 1).
                            w2_t = w2_pool.tile(
                                [P, H], f32r, name=f"w2_{k}", tag=f"w2_{k}"
                            )
                            nc.sync.dma_start(out=w2_t, in_=w2[k * P : (k + 1) * P, :])
                            w2_tiles[k] = w2_t
                        ht_t = ht_pool.tile([P, cw], f32r, name="ht_t", tag="ht")
                        nc.sync.dma_start(out=ht_t, in_=ht[k * P : (k + 1) * P, cs])
                        for h in range(KT):
                            nc.tensor.matmul(
                                ps_tiles[h],
                                lhsT=w2_tiles[k][:, h * P : (h + 1) * P],
                                rhs=ht_t,
                                start=(k == 0),
                                stop=(k == IT - 1),
                            )
                    for h in range(KT):
                        ys = ys_pool.tile([P, cw], f32, name="ys", tag="ys")
                        # Alternate ACT/DVE so the end-of-chunk PSUM drains
                        # don't serialize on one engine.
                        if h % 2 == 0:
                            nc.vector.tensor_scalar_add(
                                ys, ps_tiles[h], b2_sb[:, h : h + 1]
                            )
                        else:
                            nc.scalar.activation(
                                ys, ps_tiles[h], IDENT, bias=b2_sb[:, h : h + 1]
                            )
                        nc.sync.dma_start(out=yt[h * P : (h + 1) * P, cs], in_=ys)

    nc.compile()
    return nc


def kernel(hidden_states, gate_w, w1, b1, w2, b2):
    global LAST_EXEC_TIME_NS
    import jax
    import jax.numpy as jnp

    from concourse.bass_utils import run_bass_kernel_spmd

    # ---- gating: mirror the reference's jax ops so routing matches ----
    hs_j = jnp.asarray(hidden_states)
    xf_j = hs_j.reshape(T, H)
    logits = xf_j @ jnp.asarray(gate_w).T  # (T, E)
    top_vals, top_idx = jax.lax.top_k(logits, TOP_K)  # (T, K)
    gates = jax.nn.softmax(top_vals, axis=-1)  # (T, K)

    probs = jax.nn.softmax(logits, axis=-1)
    mean_probs = probs.mean(axis=0)
    routing_probs = (probs > 0).astype(probs.dtype).mean(axis=0)
    aux_loss = np.asarray(E * (mean_probs * routing_probs).sum())

    xf = np.asarray(xf_j, dtype=np.float32)
    top_idx_np = np.asarray(top_idx)
    gates_np = np.asarray(gates, dtype=np.float32)
    w1_np = np.asarray(w1, dtype=np.float32)
    b1_np = np.asarray(b1, dtype=np.float32)
    w2_np = np.asarray(w2, dtype=np.float32)
    b2_np = np.asarray(b2, dtype=np.float32)

    # ---- dispatch: first CAPACITY tokens (row-major order) per (k, e) ----
    rows_per_expert = []  # e -> list over k of kept token indices
    for e in range(E):
        per_k = []
        for k in range(TOP_K):
            tok = np.nonzero(top_idx_np[:, k] == e)[0]
            per_k.append(tok[:CAPACITY])
        rows_per_expert.append(per_k)

    max_rows = max(sum(len(t) for t in per_k) for per_k in rows_per_expert)
    M = max(256, ((max_rows + P - 1) // P) * P)

    in_maps = []
    for e in range(E):
        rows = np.concatenate(rows_per_expert[e])
        xt_e = np.zeros((H, M), dtype=np.float32)
        if len(rows):
            xt_e[:, : len(rows)] = xf[rows].T
        in_maps.append(
            {
                "xt": xt_e,
                "w1": np.ascontiguousarray(w1_np[e]),
                "b1": np.ascontiguousarray(b1_np[e]),
                "w2": np.ascontiguousarray(w2_np[e]),
                "b2": np.ascontiguousarray(b2_np[e]),
            }
        )

    # ---- device FFN ----
    nc = _build_bass(M)
    trace = bool(os.environ.get("MOE_KERNEL_TRACE"))
    res = run_bass_kernel_spmd(nc, in_maps, core_ids=list(range(E)), trace=trace)
    LAST_EXEC_TIME_NS = res.exec_time_ns

    # ---- combine ----
    out = np.zeros((T, H), dtype=np.float32)
    for e in range(E):
        yt_e = np.asarray(res.results[e]["yt"])  # (H, M)
        y_e = np.ascontiguousarray(yt_e.T)  # (M, H)
        ofs = 0
        for k in range(TOP_K):
            rows = rows_per_expert[e][k]
            n = len(rows)
            if n:
                out[rows] += gates_np[rows, k][:, None] * y_e[ofs : ofs + n]
            ofs += n

    return out.reshape(B, S, H), np.float32(aux_loss)


# revision 13
# speedup vs baseline: 1.2559x; 1.0221x over previous
"""MoE layer (E=8 experts, top-2, capacity 2560) on 8 TRN2 NeuronCores.

Strategy (expert-parallel, as suggested by the sharding hint):
  - Host: gating (logits / top-k / softmax / ranks) mirrored with the exact
    same jax ops as the reference so routing decisions match bitwise.
  - Host dispatch: gather each expert's kept tokens into a dense, padded,
    pre-transposed activation buffer XT (H, M); one expert per core.
  - Device (per core): YT = W2^T gelu(W1^T XT + b1) + b2, all matmuls in
    fp32r (1 cycle/row on the PE when the moving dim >= 256).
  - Host combine: scatter-add gate-weighted rows back into the output.

The device kernel is two phases, both producing transposed outputs so no
on-device transposes are needed:
  phase 1: HT[i, m] = gelu(sum_k W1[k, i]^T @ XT[k, m] + b1[i])  -> DRAM
  phase 2: YT[h, m] = sum_k W2[k, h]^T @ HT[k, m] + b2[h]        -> DRAM

Schedule notes (from NTFF profiling):
  - Token chunks are sized {512,384,256} so M only rounds up to 128 rows
    (fp32r needs a moving dim >= 256 to run at 1 cyc/row).
  - Phase-1 chunk 0 runs k-outer over 4-PSUM-bank i-groups with W1/XT DMAs
    interleaved per-k, so the PE starts ~7us in instead of waiting ~50us
    for the whole W1 load; later chunks run i-outer (gelu drains pipeline
    against the accumulation of the next i-tile).
  - Phase-2 W2 tile loads are interleaved into chunk 0's k-loop, removing
    the ~47us phase-transition stall a monolithic W2 load causes.
"""

import os

import numpy as np

B, S, H, I, E = 4, 2048, 1024, 4096, 8
TOP_K = 2
T = B * S
CAPACITY = int(1.25 * T * TOP_K / E)  # 2560
P = 128
KT = H // P  # 8 k-tiles over H
IT = I // P  # 32 i-tiles over I

# Set by kernel() after a profiled run (test harness convenience).
LAST_EXEC_TIME_NS = None


def _plan_chunks(M):
    """Split M (multiple of 128, >= 256) into chunks of 256..512 rows."""
    chunks = []
    while M > 768:
        chunks.append(512)
        M -= 512
    chunks += {256: [256], 384: [384], 512: [512], 640: [384, 256], 768: [512, 256]}[M]
    return chunks


def _build_bass(M: int):
    import concourse.bass as bass  # noqa: F401
    import concourse.mybir as mybir
    from concourse import bacc
    from concourse.tile import TileContext

    f32 = mybir.dt.float32
    f32r = mybir.dt.float32r
    IDENT = mybir.ActivationFunctionType.Identity
    # CoreSim doesn't implement Gelu; allow swapping it out for wiring tests.
    if os.environ.get("MOE_SIM_IDENTITY"):
        GELU = IDENT
    else:
        GELU = mybir.ActivationFunctionType.Gelu

    chunks = _plan_chunks(M)
    offs = [sum(chunks[:c]) for c in range(len(chunks))]

    # Bacc (not raw Bass): its lowering splits multi-sem waits that the
    # walrus codegen can't encode on a single instruction.
    nc = bacc.Bacc("TRN2", target_bir_lowering=False, debug=False, num_devices=E)
    # float32r is bit-identical to float32 (np dtype float32); typing the
    # whole matmul input chain as f32r satisfies the BIR verifier's
    # "rounded to FP32r" producer check while running the PE at 1 cyc/row.
    xt = nc.declare_dram_parameter("xt", [H, M], f32r, isOutput=False)
    w1 = nc.declare_dram_parameter("w1", [H, I], f32r, isOutput=False)
    b1 = nc.declare_dram_parameter("b1", [I], f32, isOutput=False)
    w2 = nc.declare_dram_parameter("w2", [I, H], f32r, isOutput=False)
    b2 = nc.declare_dram_parameter("b2", [H], f32, isOutput=False)
    yt = nc.declare_dram_parameter("yt", [H, M], f32, isOutput=True)
    ht = nc.dram_tensor("ht", [I, M], f32r)  # internal scratch

    GW = 4 * P  # W1 column-slice width = one 4-i-tile PSUM group

    with TileContext(nc) as tc:
        # w2_head outlives phase 1: the first 8 W2 k-tiles stream in during
        # phase 1's DMA slack so phase 2 starts with weights in hand.
        with tc.tile_pool(name="w2h_pool", bufs=1) as w2h_pool:
            w2h_tiles = [
                w2h_pool.tile([P, H], f32r, name=f"w2h_{k}", tag=f"w2h_{k}")
                for k in range(KT)
            ]
            # ---------------- phase 1: HT = gelu(W1^T X + b1) ----------------
            with (
                tc.tile_pool(name="w1_pool", bufs=1) as w1_pool,
                tc.tile_pool(name="b1_pool", bufs=1) as b1_pool,
                tc.tile_pool(name="xt_pool", bufs=2 * KT) as xt_pool,
                tc.tile_pool(name="hs_pool", bufs=6) as hs_pool,
                tc.tile_pool(name="psA_pool", bufs=1, space="PSUM") as psA_pool,
                tc.tile_pool(name="psB_pool", bufs=4, space="PSUM") as psB_pool,
            ):
                b1_sb = b1_pool.tile([P, IT], f32, tag="b1")
                nc.sync.dma_start(out=b1_sb, in_=b1[:].rearrange("(a p) -> p a", p=P))

                # Chunk 0: W1 arrives in GW-column slices, interleaved with
                # the XT k-tiles, issued in exactly the order the k-outer
                # group loop consumes them — the first matmul can start
                # after ~0.5 MB instead of ~19 MB.
                c0 = chunks[0]
                w1_tiles = [
                    w1_pool.tile([P, I], f32r, name=f"w1_{k}", tag=f"w1_{k}")
                    for k in range(KT)
                ]
                xt0_tiles = []
                for k in range(KT):
                    xt_t = xt_pool.tile([P, c0], f32r, name=f"xt_{k}", tag="xt")
                    nc.sync.dma_start(out=xt_t, in_=xt[k * P : (k + 1) * P, 0:c0])
                    xt0_tiles.append(xt_t)
                    nc.sync.dma_start(
                        out=w1_tiles[k][:, 0:GW], in_=w1[k * P : (k + 1) * P, 0:GW]
                    )
                for g in range(1, IT // 4):
                    gs = slice(g * GW, (g + 1) * GW)
                    for k in range(KT):
                        nc.sync.dma_start(
                            out=w1_tiles[k][:, gs], in_=w1[k * P : (k + 1) * P, gs]
                        )

                def gelu_store(ps, i, c, cs):
                    hs = hs_pool.tile([P, chunks[c]], f32r, name="hs", tag="hs")
                    nc.scalar.activation(hs, ps, GELU, bias=b1_sb[:, i : i + 1])
                    # Stores go out the gpsimd SWDGE queue: the sync HWDGE
                    # queue is a single FIFO, and a store queued behind a
                    # burst of weight loads stalls the hs-slot pipeline.
                    nc.gpsimd.dma_start(out=ht[i * P : (i + 1) * P, cs], in_=hs)

                # Chunk 0 compute: k-outer over 4-bank i-groups (paced by the
                # W1 stream; group-end gelu drains hide under the DMA wait).
                cs0 = slice(0, c0)
                for g in range(IT // 4):
                    ps_g = [
                        psA_pool.tile([P, c0], f32, name=f"psA_{j}", tag=f"psA_{j}")
                        for j in range(4)
                    ]
                    for k in range(KT):
                        for j in range(4):
                            i = 4 * g + j
                            nc.tensor.matmul(
                                ps_g[j],
                                lhsT=w1_tiles[k][:, i * P : (i + 1) * P],
                                rhs=xt0_tiles[k],
                                start=(k == 0),
                                stop=(k == KT - 1),
                            )
                    for j in range(4):
                        gelu_store(ps_g[j], 4 * g + j, 0, cs0)

                # Chunks 1+: i-outer, gelu pipelined against the next i-tile.
                for c in range(1, len(chunks)):
                    cw = chunks[c]
                    cs = slice(offs[c], offs[c] + cw)
                    xt_tiles = []
                    for k in range(KT):
                        xt_t = xt_pool.tile([P, cw], f32r, name=f"xt_{k}", tag="xt")
                        nc.sync.dma_start(out=xt_t, in_=xt[k * P : (k + 1) * P, cs])
                        xt_tiles.append(xt_t)
                    if c == min(2, len(chunks) - 1):
                        # Phase-1 DMA has ~2x slack per chunk by now; pull the
                        # first 8 W2 k-tiles in ahead of the phase switch.
                        for k in range(KT):
                            nc.sync.dma_start(
                                out=w2h_tiles[k], in_=w2[k * P : (k + 1) * P, :]
                            )
                    for i in range(IT):
                        ps = psB_pool.tile([P, cw], f32, name="ps", tag="psB")
                        for k in range(KT):
                            nc.tensor.matmul(
                                ps,
                                lhsT=w1_tiles[k][:, i * P : (i + 1) * P],
                                rhs=xt_tiles[k],
                                start=(k == 0),
                                stop=(k == KT - 1),
                            )
                        gelu_store(ps, i, c, cs)

                if len(chunks) == 1:
                    for k in range(KT):
                        nc.sync.dma_start(
                            out=w2h_tiles[k], in_=w2[k * P : (k + 1) * P, :]
                        )

            # ---------------- phase 2: YT = W2^T HT + b2 ----------------
            with (
                tc.tile_pool(name="w2_pool", bufs=1) as w2_pool,
                tc.tile_pool(name="b2_pool", bufs=1) as b2_pool,
                tc.tile_pool(name="ht_pool", bufs=24) as ht_pool,
                tc.tile_pool(name="ys_pool", bufs=6) as ys_pool,
                tc.tile_pool(name="ps2_pool", bufs=1, space="PSUM") as ps2_pool,
            ):
                b2_sb = b2_pool.tile([P, KT], f32, tag="b2")
                nc.sync.dma_start(out=b2_sb, in_=b2[:].rearrange("(a p) -> p a", p=P))

                w2_tiles = list(w2h_tiles) + [None] * (IT - KT)

                for c in range(len(chunks)):
                    cw = chunks[c]
                    cs = slice(offs[c], offs[c] + cw)
                    ps_tiles = [
                        ps2_pool.tile([P, cw], f32, name=f"psy_{h}", tag=f"ps2_{h}")
                        for h in range(KT)
                    ]
                    for k in range(IT):
                        if c == 0 and k >= KT:
                            # Interleave the rest of the W2 load with chunk
                            # 0's HT stream (k < KT came in during phase 1).
                            w2_t = w2_pool.tile(
                                [P, H], f32r, name=f"w2_{k}", tag=f"w2_{k}"
                            )
                            nc.sync.dma_start(out=w2_t, in_=w2[k * P : (k + 1) * P, :])
                            w2_tiles[k] = w2_t
                        ht_t = ht_pool.tile([P, cw], f32r, name="ht_t", tag="ht")
                        nc.sync.dma_start(out=ht_t, in_=ht[k * P : (k + 1) * P, cs])
                        for h in range(KT):
                            nc.tensor.matmul(
                                ps_tiles[h],
                                lhsT=w2_tiles[k][:, h * P : (h + 1) * P],
                                rhs=ht_t,
                                start=(k == 0),
                                stop=(k == IT - 1),
                            )
                    for h in range(KT):
                        ys = ys_pool.tile([P, cw], f32, name="ys", tag="ys")
                        # Alternate ACT/DVE so the end-of-chunk PSUM drains
                        # don't serialize on one engine.
                        if h % 2 == 0:
                            nc.vector.tensor_scalar_add(
                                ys, ps_tiles[h], b2_sb[:, h : h + 1]
                            )
                        else:
                            nc.scalar.activation(
                                ys, ps_tiles[h], IDENT, bias=b2_sb[:, h : h + 1]
                            )
                        nc.gpsimd.dma_start(out=yt[h * P : (h + 1) * P, cs], in_=ys)

    nc.compile()
    return nc


def kernel(hidden_states, gate_w, w1, b1, w2, b2):
    global LAST_EXEC_TIME_NS
    import jax
    import jax.numpy as jnp

    from concourse.bass_utils import run_bass_kernel_spmd

    # ---- gating: mirror the reference's jax ops so routing matches ----
    hs_j = jnp.asarray(hidden_states)
    xf_j = hs_j.reshape(T, H)
    logits = xf_j @ jnp.asarray(gate_w).T  # (T, E)
    top_vals, top_idx = jax.lax.top_k(logits, TOP_K)  # (T, K)
    gates = jax.nn.softmax(top_vals, axis=-1)  # (T, K)

    probs = jax.nn.softmax(logits, axis=-1)
    mean_probs = probs.mean(axis=0)
    routing_probs = (probs > 0).astype(probs.dtype).mean(axis=0)
    aux_loss = np.asarray(E * (mean_probs * routing_probs).sum())

    xf = np.asarray(xf_j, dtype=np.float32)
    top_idx_np = np.asarray(top_idx)
    gates_np = np.asarray(gates, dtype=np.float32)
    w1_np = np.asarray(w1, dtype=np.float32)
    b1_np = np.asarray(b1, dtype=np.float32)
    w2_np = np.asarray(w2, dtype=np.float32)
    b2_np = np.asarray(b2, dtype=np.float32)

    # ---- dispatch: first CAPACITY tokens (row-major order) per (k, e) ----
    rows_per_expert = []  # e -> list over k of kept token indices
    for e in range(E):
        per_k = []
        for k in range(TOP_K):
            tok = np.nonzero(top_idx_np[:, k] == e)[0]
            per_k.append(tok[:CAPACITY])
        rows_per_expert.append(per_k)

    max_rows = max(sum(len(t) for t in per_k) for per_k in rows_per_expert)
    M = max(256, ((max_rows + P - 1) // P) * P)

    in_maps = []
    for e in range(E):
        rows = np.concatenate(rows_per_expert[e])
        xt_e = np.zeros((H, M), dtype=np.float32)
        if len(rows):
            xt_e[:, : len(rows)] = xf[rows].T
        in_maps.append(
            {
                "xt": xt_e,
                "w1": np.ascontiguousarray(w1_np[e]),
                "b1": np.ascontiguousarray(b1_np[e]),
                "w2": np.ascontiguousarray(w2_np[e]),
                "b2": np.ascontiguousarray(b2_np[e]),
            }
        )

    # ---- device FFN ----
    nc = _build_bass(M)
    trace = bool(os.environ.get("MOE_KERNEL_TRACE"))
    res = run_bass_kernel_spmd(nc, in_maps, core_ids=list(range(E)), trace=trace)
    LAST_EXEC_TIME_NS = res.exec_time_ns

    # ---- combine ----
    out = np.zeros((T, H), dtype=np.float32)
    for e in range(E):
        yt_e = np.asarray(res.results[e]["yt"])  # (H, M)
        y_e = np.ascontiguousarray(yt_e.T)  # (M, H)
        ofs = 0
        for k in range(TOP_K):
            rows = rows_per_expert[e][k]
            n = len(rows)
            if n:
                out[rows] += gates_np[rows, k][:, None] * y_e[ofs : ofs + n]
            ofs += n

    return out.reshape(B, S, H), np.float32(aux_loss)


# revision 15
# speedup vs baseline: 1.2808x; 1.0198x over previous
"""MoE layer (E=8 experts, top-2, capacity 2560) on 8 TRN2 NeuronCores.

Strategy (expert-parallel, as suggested by the sharding hint):
  - Host: gating (logits / top-k / softmax / ranks) mirrored with the exact
    same jax ops as the reference so routing decisions match bitwise.
  - Host dispatch: gather each expert's kept tokens into a dense, padded,
    pre-transposed activation buffer XT (H, M); one expert per core.
  - Device (per core): YT = W2^T gelu(W1^T XT + b1) + b2, all matmuls in
    fp32r (1 cycle/row on the PE when the moving dim >= 256).
  - Host combine: scatter-add gate-weighted rows back into the output.

The device kernel is two phases, both producing transposed outputs so no
on-device transposes are needed:
  phase 1: HT[i, m] = gelu(sum_k W1[k, i]^T @ XT[k, m] + b1[i])  -> DRAM
  phase 2: YT[h, m] = sum_k W2[k, h]^T @ HT[k, m] + b2[h]        -> DRAM

Schedule notes (from NTFF profiling):
  - Token chunks are sized {512,384,256} so M only rounds up to 128 rows
    (fp32r needs a moving dim >= 256 to run at 1 cyc/row).
  - Phase-1 chunk 0 runs k-outer over 4-PSUM-bank i-groups with W1/XT DMAs
    interleaved per-k, so the PE starts ~7us in instead of waiting ~50us
    for the whole W1 load; later chunks run i-outer (gelu drains pipeline
    against the accumulation of the next i-tile).
  - Phase-2 W2 tile loads are interleaved into chunk 0's k-loop, removing
    the ~47us phase-transition stall a monolithic W2 load causes.
"""

import os

import numpy as np

B, S, H, I, E = 4, 2048, 1024, 4096, 8
TOP_K = 2
T = B * S
CAPACITY = int(1.25 * T * TOP_K / E)  # 2560
P = 128
KT = H // P  # 8 k-tiles over H
IT = I // P  # 32 i-tiles over I

# Set by kernel() after a profiled run (test harness convenience).
LAST_EXEC_TIME_NS = None


def _plan_chunks(M):
    """Split M (multiple of 128, >= 256) into chunks of 256..512 rows."""
    chunks = []
    while M > 768:
        chunks.append(512)
        M -= 512
    chunks += {256: [256], 384: [384], 512: [512], 640: [384, 256], 768: [512, 256]}[M]
    return chunks


def _build_bass(M: int):
    import concourse.bass as bass  # noqa: F401
    import concourse.mybir as mybir
    from concourse import bacc
    from concourse.tile import TileContext

    f32 = mybir.dt.float32
    f32r = mybir.dt.float32r
    IDENT = mybir.ActivationFunctionType.Identity
    # CoreSim doesn't implement Gelu; allow swapping it out for wiring tests.
    if os.environ.get("MOE_SIM_IDENTITY"):
        GELU = IDENT
    else:
        GELU = mybir.ActivationFunctionType.Gelu

    chunks = _plan_chunks(M)
    offs = [sum(chunks[:c]) for c in range(len(chunks))]

    # Bacc (not raw Bass): its lowering splits multi-sem waits that the
    # walrus codegen can't encode on a single instruction.
    nc = bacc.Bacc("TRN2", target_bir_lowering=False, debug=False, num_devices=E)
    # float32r is bit-identical to float32 (np dtype float32); typing the
    # whole matmul input chain as f32r satisfies the BIR verifier's
    # "rounded to FP32r" producer check while running the PE at 1 cyc/row.
    xt = nc.declare_dram_parameter("xt", [H, M], f32r, isOutput=False)
    w1 = nc.declare_dram_parameter("w1", [H, I], f32r, isOutput=False)
    b1 = nc.declare_dram_parameter("b1", [I], f32, isOutput=False)
    w2 = nc.declare_dram_parameter("w2", [I, H], f32r, isOutput=False)
    b2 = nc.declare_dram_parameter("b2", [H], f32, isOutput=False)
    yt = nc.declare_dram_parameter("yt", [H, M], f32, isOutput=True)
    ht = nc.dram_tensor("ht", [I, M], f32r)  # internal scratch

    GW = 4 * P  # W1 column-slice width = one 4-i-tile PSUM group

    with TileContext(nc) as tc:
        # w2_head outlives phase 1: the first 8 W2 k-tiles stream in during
        # phase 1's DMA slack so phase 2 starts with weights in hand.
        with tc.tile_pool(name="w2h_pool", bufs=1) as w2h_pool:
            w2h_tiles = [
                w2h_pool.tile([P, H], f32r, name=f"w2h_{k}", tag=f"w2h_{k}")
                for k in range(KT)
            ]
            # ---------------- phase 1: HT = gelu(W1^T X + b1) ----------------
            with (
                tc.tile_pool(name="w1_pool", bufs=1) as w1_pool,
                tc.tile_pool(name="b1_pool", bufs=1) as b1_pool,
                tc.tile_pool(name="xt_pool", bufs=2 * KT) as xt_pool,
                tc.tile_pool(name="hs_pool", bufs=6) as hs_pool,
                tc.tile_pool(name="psA_pool", bufs=1, space="PSUM") as psA_pool,
                tc.tile_pool(name="psB_pool", bufs=4, space="PSUM") as psB_pool,
            ):
                b1_sb = b1_pool.tile([P, IT], f32, tag="b1")
                nc.sync.dma_start(out=b1_sb, in_=b1[:].rearrange("(a p) -> p a", p=P))

                # Chunk 0: W1 arrives in GW-column slices, interleaved with
                # the XT k-tiles, issued in exactly the order the k-outer
                # group loop consumes them — the first matmul can start
                # after ~0.5 MB instead of ~19 MB.
                c0 = chunks[0]
                w1_tiles = [
                    w1_pool.tile([P, I], f32r, name=f"w1_{k}", tag=f"w1_{k}")
                    for k in range(KT)
                ]
                xt0_tiles = []
                for k in range(KT):
                    xt_t = xt_pool.tile([P, c0], f32r, name=f"xt_{k}", tag="xt")
                    nc.sync.dma_start(out=xt_t, in_=xt[k * P : (k + 1) * P, 0:c0])
                    xt0_tiles.append(xt_t)
                    nc.sync.dma_start(
                        out=w1_tiles[k][:, 0:GW], in_=w1[k * P : (k + 1) * P, 0:GW]
                    )
                for g in range(1, IT // 4):
                    gs = slice(g * GW, (g + 1) * GW)
                    for k in range(KT):
                        nc.sync.dma_start(
                            out=w1_tiles[k][:, gs], in_=w1[k * P : (k + 1) * P, gs]
                        )

                def gelu_store(ps, i, c, cs):
                    hs = hs_pool.tile([P, chunks[c]], f32r, name="hs", tag="hs")
                    nc.scalar.activation(hs, ps, GELU, bias=b1_sb[:, i : i + 1])
                    # Stores go out the gpsimd SWDGE queue: the sync HWDGE
                    # queue is a single FIFO, and a store queued behind a
                    # burst of weight loads stalls the hs-slot pipeline.
                    nc.gpsimd.dma_start(out=ht[i * P : (i + 1) * P, cs], in_=hs)

                # Chunk 0 compute: k-outer over 4-bank i-groups (paced by the
                # W1 stream; group-end gelu drains hide under the DMA wait).
                cs0 = slice(0, c0)
                for g in range(IT // 4):
                    ps_g = [
                        psA_pool.tile([P, c0], f32, name=f"psA_{j}", tag=f"psA_{j}")
                        for j in range(4)
                    ]
                    for k in range(KT):
                        for j in range(4):
                            i = 4 * g + j
                            nc.tensor.matmul(
                                ps_g[j],
                                lhsT=w1_tiles[k][:, i * P : (i + 1) * P],
                                rhs=xt0_tiles[k],
                                start=(k == 0),
                                stop=(k == KT - 1),
                            )
                    for j in range(4):
                        gelu_store(ps_g[j], 4 * g + j, 0, cs0)

                # Chunks 1+: i-outer, gelu pipelined against the next i-tile.
                for c in range(1, len(chunks)):
                    cw = chunks[c]
                    cs = slice(offs[c], offs[c] + cw)
                    xt_tiles = []
                    for k in range(KT):
                        xt_t = xt_pool.tile([P, cw], f32r, name=f"xt_{k}", tag="xt")
                        nc.sync.dma_start(out=xt_t, in_=xt[k * P : (k + 1) * P, cs])
                        xt_tiles.append(xt_t)
                    if c == min(2, len(chunks) - 1):
                        # Phase-1 DMA has ~2x slack per chunk by now; pull the
                        # first 8 W2 k-tiles in ahead of the phase switch.
                        for k in range(KT):
                            nc.sync.dma_start(
                                out=w2h_tiles[k], in_=w2[k * P : (k + 1) * P, :]
                            )
                    for i in range(IT):
                        ps = psB_pool.tile([P, cw], f32, name="ps", tag="psB")
                        for k in range(KT):
                            nc.tensor.matmul(
                                ps,
                                lhsT=w1_tiles[k][:, i * P : (i + 1) * P],
                                rhs=xt_tiles[k],
                                start=(k == 0),
                                stop=(k == KT - 1),
                            )
                        gelu_store(ps, i, c, cs)

                if len(chunks) == 1:
                    for k in range(KT):
                        nc.sync.dma_start(
                            out=w2h_tiles[k], in_=w2[k * P : (k + 1) * P, :]
                        )

            # ---------------- phase 2: YT = W2^T HT + b2 ----------------
            with (
                tc.tile_pool(name="w2_pool", bufs=1) as w2_pool,
                tc.tile_pool(name="b2_pool", bufs=1) as b2_pool,
                tc.tile_pool(name="ht_pool", bufs=24) as ht_pool,
                tc.tile_pool(name="ys_pool", bufs=8) as ys_pool,
                tc.tile_pool(name="ps2_pool", bufs=1, space="PSUM") as ps2_pool,
            ):
                b2_sb = b2_pool.tile([P, KT], f32, tag="b2")
                nc.sync.dma_start(out=b2_sb, in_=b2[:].rearrange("(a p) -> p a", p=P))

                w2_tiles = list(w2h_tiles) + [None] * (IT - KT)

                for c in range(len(chunks)):
                    cw = chunks[c]
                    cs = slice(offs[c], offs[c] + cw)
                    ps_tiles = [
                        ps2_pool.tile([P, cw], f32, name=f"psy_{h}", tag=f"ps2_{h}")
                        for h in range(KT)
                    ]
                    for k in range(IT):
                        if c == 0 and k >= KT:
                            # Interleave the rest of the W2 load with chunk
                            # 0's HT stream (k < KT came in during phase 1).
                            w2_t = w2_pool.tile(
                                [P, H], f32r, name=f"w2_{k}", tag=f"w2_{k}"
                            )
                            nc.sync.dma_start(out=w2_t, in_=w2[k * P : (k + 1) * P, :])
                            w2_tiles[k] = w2_t
                        ht_t = ht_pool.tile([P, cw], f32r, name="ht_t", tag="ht")
                        nc.sync.dma_start(out=ht_t, in_=ht[k * P : (k + 1) * P, cs])
                        for h in range(KT):
                            nc.tensor.matmul(
                                ps_tiles[h],
                                lhsT=w2_tiles[k][:, h * P : (h + 1) * P],
                                rhs=ht_t,
                                start=(k == 0),
                                stop=(k == IT - 1),
                            )
                    for h in range(KT):
                        ys = ys_pool.tile([P, cw], f32, name="ys", tag="ys")
                        # Alternate ACT/DVE so the end-of-chunk PSUM drains
                        # don't serialize on one engine.
                        if h % 2 == 0:
                            nc.vector.tensor_scalar_add(
                                ys, ps_tiles[h], b2_sb[:, h : h + 1]
                            )
                        else:
                            nc.scalar.activation(
                                ys, ps_tiles[h], IDENT, bias=b2_sb[:, h : h + 1]
                            )
                        # yt stores ride the sync HWDGE queue: phase-2 loads
                        # have ~10x slack, and keeping SWDGE empty at kernel
                        # end avoids a ~9us GpSimd drain in the tail.
                        nc.sync.dma_start(out=yt[h * P : (h + 1) * P, cs], in_=ys)

    nc.compile()
    return nc


def kernel(hidden_states, gate_w, w1, b1, w2, b2):
    global LAST_EXEC_TIME_NS
    import jax
    import jax.numpy as jnp

    from concourse.bass_utils import run_bass_kernel_spmd

    # ---- gating: mirror the reference's jax ops so routing matches ----
    hs_j = jnp.asarray(hidden_states)
    xf_j = hs_j.reshape(T, H)
    logits = xf_j @ jnp.asarray(gate_w).T  # (T, E)
    top_vals, top_idx = jax.lax.top_k(logits, TOP_K)  # (T, K)
    gates = jax.nn.softmax(top_vals, axis=-1)  # (T, K)

    probs = jax.nn.softmax(logits, axis=-1)
    mean_probs = probs.mean(axis=0)
    routing_probs = (probs > 0).astype(probs.dtype).mean(axis=0)
    aux_loss = np.asarray(E * (mean_probs * routing_probs).sum())

    xf = np.asarray(xf_j, dtype=np.float32)
    top_idx_np = np.asarray(top_idx)
    gates_np = np.asarray(gates, dtype=np.float32)
    w1_np = np.asarray(w1, dtype=np.float32)
    b1_np = np.asarray(b1, dtype=np.float32)
    w2_np = np.asarray(w2, dtype=np.float32)
    b2_np = np.asarray(b2, dtype=np.float32)

    # ---- dispatch: first CAPACITY tokens (row-major order) per (k, e) ----
    rows_per_expert = []  # e -> list over k of kept token indices
    for e in range(E):
        per_k = []
        for k in range(TOP_K):
            tok = np.nonzero(top_idx_np[:, k] == e)[0]
            per_k.append(tok[:CAPACITY])
        rows_per_expert.append(per_k)

    max_rows = max(sum(len(t) for t in per_k) for per_k in rows_per_expert)
    M = max(256, ((max_rows + P - 1) // P) * P)

    in_maps = []
    for e in range(E):
        rows = np.concatenate(rows_per_expert[e])
        xt_e = np.zeros((H, M), dtype=np.float32)
        if len(rows):
            xt_e[:, : len(rows)] = xf[rows].T
        in_maps.append(
            {
                "xt": xt_e,
                "w1": np.ascontiguousarray(w1_np[e]),
                "b1": np.ascontiguousarray(b1_np[e]),
                "w2": np.ascontiguousarray(w2_np[e]),
                "b2": np.ascontiguousarray(b2_np[e]),
            }
        )

    # ---- device FFN ----
    nc = _build_bass(M)
    trace = bool(os.environ.get("MOE_KERNEL_TRACE"))
    res = run_bass_kernel_spmd(nc, in_maps, core_ids=list(range(E)), trace=trace)
    LAST_EXEC_TIME_NS = res.exec_time_ns

    # ---- combine ----
    out = np.zeros((T, H), dtype=np.float32)
    for e in range(E):
        yt_e = np.asarray(res.results[e]["yt"])  # (H, M)
        y_e = np.ascontiguousarray(yt_e.T)  # (M, H)
        ofs = 0
        for k in range(TOP_K):
            rows = rows_per_expert[e][k]
            n = len(rows)
            if n:
                out[rows] += gates_np[rows, k][:, None] * y_e[ofs : ofs + n]
            ofs += n

    return out.reshape(B, S, H), np.float32(aux_loss)
